# revision 32
# baseline (speedup 1.0000x reference)
"""Trainium2 Bass kernel for AllGNN message passing.

Computes, for full inputs:
    h   = x @ W_in + b_in
    deg = adj.sum(axis=1, keepdims=True)
    agg = (adj @ h) / (deg + 1)
    out = agg @ W_cls + b_cls

Key algebra: row scaling commutes with the right matmul, so
    out = (adj @ G)[:, :C] / (deg+1) + b_cls
with G = [x @ W2 + b2 | ones], W2 = W_in @ W_cls, b2 = b_in @ W_cls.
The ones column's product recovers deg. The ones column is folded into the
G build by padding W2 with a zero column and b2 with a one.

Sharding: row-shard adj over 8 cores; x is shipped pre-transposed (and
pre-cast to bf16) so each core computes the full G locally with W2 as the
stationary matmul operand — no collectives at all. Each core then streams
its adj row-block once from HBM (SWDGE fp32->bf16 cast; adj is 0/1 so bf16
is exact), transposes 128x128 blocks on the PE (is_transpose against
identity), and accumulates out.T = G.T @ adj.T chunk-by-chunk in PSUM with
G tiles as the stationary operand. Group 0's accumulation is deferred a
couple of chunks so G is ready before the first accumulating matmul.
"""

import numpy as np

import concourse.bass as bass
from concourse import bacc
import concourse.mybir as mybir
import concourse.tile as tile
from concourse.bass_utils import run_bass_kernel_spmd

import ml_dtypes

N_CORES = 8
N_NODES = 12000
IN_CH = 256
HID = 64
N_CLS = 40

JW = 128  # j (contraction) tile width
IW = 128  # i (output-row) tile width
XC = 512  # x/g chunk width (columns of g.T per matmul)


def _ceil_div(a, b):
    return -(-a // b)


def build_gnn(
    n_nodes=N_NODES,
    n_cores=N_CORES,
    in_ch=IN_CH,
    hid=HID,
    n_cls=N_CLS,
    stage_jtiles=12,
    group_its=4,
    strip_bufs=None,
    nat_bufs=None,
    act_copy_every=2,
    use_is_transpose=True,
    chain0_delay=4,
):
    f32 = mybir.dt.float32
    bf16 = mybir.dt.bfloat16
    mult = mybir.AluOpType.mult
    add = mybir.AluOpType.add

    assert n_nodes % n_cores == 0
    rows = n_nodes // n_cores
    assert in_ch % 128 == 0
    n_kt = in_ch // 128
    assert hid <= 128 and n_cls + 1 <= 128
    gc = n_cls + 1  # G columns: [g | ones]

    n_jt = _ceil_div(n_nodes, JW)
    n_it = _ceil_div(rows, IW)
    stage_cols = stage_jtiles * JW
    n_chunks = _ceil_div(n_nodes, stage_cols)
    n_xc = _ceil_div(n_nodes, XC)

    nc = bacc.Bacc(num_devices=n_cores)

    adj_h = nc.dram_tensor("adj_blk", [rows, n_nodes], f32, kind="ExternalInput")
    xt_h = nc.dram_tensor("x_Ti", [128, n_xc, n_kt, XC], bf16, kind="ExternalInput")
    win_h = nc.dram_tensor("W_in", [in_ch, hid], f32, kind="ExternalInput")
    bin_h = nc.dram_tensor("b_in", [hid], f32, kind="ExternalInput")
    wcls_h = nc.dram_tensor("W_cls", [hid, n_cls], f32, kind="ExternalInput")
    bcls_h = nc.dram_tensor("b_cls", [n_cls], f32, kind="ExternalInput")
    out_h = nc.dram_tensor("out_blk", [rows, n_cls], f32, kind="ExternalOutput")

    id_f_dram = nc.inline_tensor(np.eye(128, dtype=np.float32), name="ident_f32")
    id_b_dram = nc.inline_tensor(
        np.eye(128).astype(ml_dtypes.bfloat16), name="ident_bf16"
    )

    n_groups = _ceil_div(n_it, group_its)
    sbufs = (
        (min(n_chunks, chain0_delay + 1) + 2) * stage_jtiles + 8
        if strip_bufs is None
        else strip_bufs
    )
    nbufs = 5 * group_its if nat_bufs is None else nat_bufs
    group_w_max = min(group_its * IW, 512)
    assert group_its * IW <= 512
    pt_dt = bf16 if use_is_transpose else f32

    with tile.TileContext(nc) as tc:
        with (
            tc.tile_pool(name="singles", bufs=1) as singles,
            tc.tile_pool(name="nat", bufs=nbufs) as nat_pool,
            tc.tile_pool(name="strip", bufs=sbufs) as strip_pool,
            tc.tile_pool(name="outp", bufs=3) as out_pool,
        ):
            id_f = singles.tile([128, 128], f32, tag="id_f")
            nc.sync.dma_start(out=id_f, in_=id_f_dram[:])
            id_b = singles.tile([128, 128], bf16, tag="id_b")
            nc.sync.dma_start(out=id_b, in_=id_b_dram[:])
            # b_cls broadcast across partitions
            bcls_sb = singles.tile([128, n_cls], f32, tag="bcls")
            bc = bcls_h[:]
            nc.gpsimd.dma_start(
                out=bcls_sb,
                in_=bass.AP(tensor=bc.tensor, offset=bc.offset, ap=[[0, 128]] + bc.ap),
            )
            G_sb = singles.tile([128, n_jt, gc], bf16, tag="G")

            # ---- Phase A: G = [x @ W2 + b2 | 1] computed fully per core ----
            with (
                tc.tile_pool(name="ph_a", bufs=3) as pa,
                tc.tile_pool(name="psc", bufs=1, space="PSUM") as psc,
            ):
                psb = psc
                win_sb = pa.tile([128, n_kt, hid], f32, tag="win")
                nc.sync.dma_start(
                    out=win_sb, in_=win_h[:].rearrange("(t p) h -> p t h", p=128)
                )
                wcls_sb = pa.tile([hid, n_cls], f32, tag="wcls")
                nc.sync.dma_start(out=wcls_sb, in_=wcls_h[:])
                bin_sb = pa.tile([hid, 1], f32, tag="bin")
                bi = bin_h[:]
                nc.sync.dma_start(
                    out=bin_sb,
                    in_=bass.AP(
                        tensor=bi.tensor, offset=bi.offset, ap=bi.ap + [[0, 1]]
                    ),
                )

                ones_sb = pa.tile([1, 128], f32, tag="ones")
                nc.vector.memset(ones_sb, 1.0)
                # G ones column (deg recovery), written once
                nc.vector.memset(G_sb[:, :, n_cls:gc], 1.0)

                # W_in.T tiles via PE transpose (fp32)
                winT_sb = pa.tile([hid, n_kt, 128], f32, tag="winT")
                for t in range(n_kt):
                    ps_w = psb.tile([128, 512], f32, tag="acc", name="acc", bufs=4)
                    ps = ps_w[:hid, :128]
                    nc.tensor.matmul(
                        ps, lhsT=win_sb[:, t, :], rhs=id_f, start=True, stop=True
                    )
                    nc.vector.tensor_copy(winT_sb[:, t, :], ps)
                # W2 = W_in @ W_cls -> bf16
                w2b_sb = pa.tile([128, n_kt, n_cls], bf16, tag="w2b")
                for t in range(n_kt):
                    ps_w = psb.tile([128, 512], f32, tag="acc", name="acc", bufs=4)
                    ps = ps_w[:, :n_cls]
                    nc.tensor.matmul(
                        ps, lhsT=winT_sb[:, t, :], rhs=wcls_sb, start=True, stop=True
                    )
                    nc.vector.tensor_copy(w2b_sb[:, t, :], ps)
                # b2 = b_in @ W_cls broadcast to [128, n_cls]
                ps_b2w = psb.tile([128, 512], f32, tag="acc", name="acc", bufs=4)
                ps_b2 = ps_b2w[:1, :n_cls]
                nc.tensor.matmul(ps_b2, lhsT=bin_sb, rhs=wcls_sb, start=True, stop=True)
                b2row = pa.tile([1, n_cls], f32, tag="b2row")
                nc.vector.tensor_copy(b2row, ps_b2)
                ps_b2bw = psb.tile([128, 512], f32, tag="acc", name="acc", bufs=4)
                ps_b2b = ps_b2bw[:, :n_cls]
                nc.tensor.matmul(ps_b2b, lhsT=ones_sb, rhs=b2row, start=True, stop=True)
                b2b_sb = pa.tile([128, n_cls], f32, tag="b2b")
                nc.vector.tensor_copy(b2b_sb, ps_b2b)

                # g = x @ W2 + b2, with pre-transposed x slices as the
                # stationary operand, written straight into G_sb node tiles.
                # Emitted as jobs interleaved into group 0's streaming chunks
                # so phase A's PE work doesn't starve the transpose pipeline.
                def g_job(ch):
                    c0 = ch * XC
                    cw = min(XC, n_nodes - c0)
                    xts = pa.tile([128, n_kt, XC], bf16, tag="xts", name="xts")
                    nc.sync.dma_start(out=xts, in_=xt_h[:, ch, :, :])
                    for q in range(_ceil_div(cw, JW)):
                        qw = min(JW, cw - q * JW)
                        jt = (c0 + q * JW) // JW
                        ps_gw = psb.tile(
                            [128, 512], f32, tag="acc", name="acc", bufs=4
                        )
                        ps_g = ps_gw[:, :n_cls]
                        for t in range(n_kt):
                            nc.tensor.matmul(
                                ps_g[:qw, :],
                                lhsT=xts[:, t, q * JW : q * JW + qw],
                                rhs=w2b_sb[:, t, :],
                                start=(t == 0),
                                stop=(t == n_kt - 1),
                            )
                        nc.vector.tensor_add(
                            G_sb[:qw, jt, 0:n_cls], ps_g[:qw, :], b2b_sb[:qw]
                        )

                g_jobs = list(range(n_xc))
                g_spread = max(1, min(n_chunks - 1, chain0_delay))
                g_per = _ceil_div(n_xc, g_spread)

                # -- Phase B: stream adj, transpose on PE, accumulate out.T --
                psc = psb
                copy_state = [0]

                class AccChain:
                    """One group's out.T accumulation: even/odd j-tiles go to
                    two PSUM banks so consecutive matmuls pipeline."""

                    NWAY = 3

                    def __init__(self, grp_info):
                        self.grp_info = grp_info
                        self.next_jt = 0
                        self.ps = [
                            psc.tile(
                                [gc, group_w_max], f32, tag="acc", name="acc", bufs=4
                            )
                            for _ in range(self.NWAY)
                        ]

                    def feed(self, upto_jt):
                        its, widths, offs, gw, strips = self.grp_info
                        while self.next_jt < upto_jt:
                            jt = self.next_jt
                            jw = min(JW, n_nodes - jt * JW)
                            ps = self.ps[jt % self.NWAY]
                            nc.tensor.matmul(
                                ps[:, :gw],
                                lhsT=G_sb[:jw, jt, :],
                                rhs=strips[jt][:jw, :gw],
                                start=(jt < self.NWAY),
                                stop=(jt >= n_jt - self.NWAY),
                                skip_group_check=True,
                            )
                            self.next_jt += 1
                        if self.next_jt == n_jt:
                            self._finalize()
                            return True
                        return False

                    def _finalize(self):
                        its, widths, offs, gw, strips = self.grp_info
                        U_sb = out_pool.tile([gc, group_w_max], f32, tag="U")
                        nc.vector.tensor_copy(U_sb[:, :gw], self.ps[0][:, :gw])
                        for w in range(1, self.NWAY):
                            nc.vector.tensor_add(
                                U_sb[:, :gw], U_sb[:, :gw], self.ps[w][:, :gw]
                            )
                        for k, it in enumerate(its):
                            i0 = it * IW
                            p = widths[k]
                            go = offs[k]
                            ps_f = psc.tile(
                                [128, gc], f32, tag="fin", name="fin", bufs=1
                            )
                            nc.tensor.matmul(
                                ps_f[:p, :],
                                lhsT=U_sb[:, go : go + p],
                                rhs=id_f[:gc, :gc],
                                start=True,
                                stop=True,
                                skip_group_check=True,
                            )
                            deg1 = out_pool.tile([128, 1], f32, tag="deg1")
                            nc.vector.tensor_scalar_add(
                                deg1[:p], ps_f[:p, n_cls:gc], 1.0
                            )
                            rcp = out_pool.tile([128, 1], f32, tag="rcp")
                            nc.vector.reciprocal(rcp[:p], deg1[:p])
                            o_sb = out_pool.tile([128, n_cls], f32, tag="o")
                            nc.vector.scalar_tensor_tensor(
                                out=o_sb[:p],
                                in0=ps_f[:p, 0:n_cls],
                                scalar=rcp[:p],
                                in1=bcls_sb[:p],
                                op0=mult,
                                op1=add,
                            )
                            nc.sync.dma_start(out=out_h[i0 : i0 + p, :], in_=o_sb[:p])

                def emit_chunk(grp_info, c):
                    its, widths, offs, gw, strips = grp_info
                    j0 = c * stage_cols
                    cw = min(stage_cols, n_nodes - j0)
                    nat_tiles = []
                    for k, it in enumerate(its):
                        i0 = it * IW
                        p = widths[k]
                        nt_ = nat_pool.tile([128, stage_cols], bf16, tag="nat")
                        nc.gpsimd.dma_start(
                            out=nt_[:p, :cw], in_=adj_h[i0 : i0 + p, j0 : j0 + cw]
                        )
                        nat_tiles.append(nt_)
                    jt_lo = c * stage_jtiles
                    jt_hi = min((c + 1) * stage_jtiles, n_jt)
                    for jt in range(jt_lo, jt_hi):
                        jw = min(JW, n_nodes - jt * JW)
                        off = jt * JW - j0
                        ps = psc.tile(
                            [128, group_w_max], pt_dt, tag="pt", name="pt", bufs=3
                        )
                        for k in range(len(its)):
                            p = widths[k]
                            go = offs[k]
                            if use_is_transpose:
                                nc.tensor.matmul(
                                    ps[:jw, go : go + p],
                                    lhsT=nat_tiles[k][:p, off : off + jw],
                                    rhs=id_b[:p, :p],
                                    is_transpose=True,
                                )
                            else:
                                nc.tensor.matmul(
                                    ps[:jw, go : go + p],
                                    lhsT=nat_tiles[k][:p, off : off + jw],
                                    rhs=id_b[:p, :p],
                                    start=True,
                                    stop=True,
                                )
                        st = strips[jt]
                        if act_copy_every == 5:
                            use_act = copy_state[0] % 5 in (1, 3)
                        else:
                            use_act = act_copy_every and copy_state[0] % act_copy_every == (
                                act_copy_every - 1
                            )
                        if use_act:
                            nc.scalar.copy(st[:jw, :gw], ps[:jw, :gw])
                        else:
                            nc.vector.tensor_copy(st[:jw, :gw], ps[:jw, :gw])
                        copy_state[0] += 1
                    return jt_hi

                # every group accumulates its own strips within each chunk;
                # group 0 defers by `chain0_delay` chunks so G (phase A) is
                # ready before the first accumulating matmul.
                for grp in range(n_groups):
                    its = list(range(grp * group_its, min((grp + 1) * group_its, n_it)))
                    widths = [min(IW, rows - it * IW) for it in its]
                    offs = [sum(widths[:k]) for k in range(len(its))]
                    gw = sum(widths)
                    strips = [
                        strip_pool.tile(
                            [128, group_w_max], bf16, tag="strip", name="strip"
                        )
                        for _ in range(n_jt)
                    ]
                    grp_info = (its, widths, offs, gw, strips)
                    chain = None
                    delay = chain0_delay if grp == 0 else 0
                    for c in range(n_chunks):
                        jt_hi = emit_chunk(grp_info, c)
                        # interleave phase A's g computation into group 0's
                        # first chunks (keeps the PE fed without a long
                        # serial phase A before streaming starts)
                        for _ in range(g_per):
                            if g_jobs:
                                g_job(g_jobs.pop(0))
                        if chain is None and c >= delay and not g_jobs:
                            chain = AccChain(grp_info)
                        if chain is not None:
                            chain.feed(jt_hi)

    nc.compile()
    return nc


_CACHE = {}


def _get_nc():
    if "nc" not in _CACHE:
        _CACHE["nc"] = build_gnn()
    return _CACHE["nc"]


def make_in_maps(x, adj, W_in, b_in, W_cls, b_cls):
    rows = adj.shape[0] // N_CORES
    n = x.shape[0]
    n_kt = x.shape[1] // 128
    n_xc = _ceil_div(n, XC)
    xp = np.zeros((x.shape[1], n_xc * XC), dtype=np.float32)
    xp[:, :n] = np.asarray(x, dtype=np.float32).T
    x_Ti = np.ascontiguousarray(
        xp.reshape(n_kt, 128, n_xc, XC).transpose(1, 2, 0, 3)
    ).astype(ml_dtypes.bfloat16)
    in_maps = []
    for c in range(N_CORES):
        sl = slice(c * rows, (c + 1) * rows)
        in_maps.append(
            {
                "adj_blk": np.ascontiguousarray(adj[sl]),
                "x_Ti": x_Ti,
                "W_in": W_in,
                "b_in": b_in,
                "W_cls": W_cls,
                "b_cls": b_cls,
            }
        )
    return in_maps


def kernel(x, adj, W_in, b_in, W_cls, b_cls):
    x = np.asarray(x, dtype=np.float32)
    adj = np.asarray(adj, dtype=np.float32)
    W_in = np.asarray(W_in, dtype=np.float32)
    b_in = np.asarray(b_in, dtype=np.float32)
    W_cls = np.asarray(W_cls, dtype=np.float32)
    b_cls = np.asarray(b_cls, dtype=np.float32)

    nc = _get_nc()
    in_maps = make_in_maps(x, adj, W_in, b_in, W_cls, b_cls)
    res = run_bass_kernel_spmd(nc, in_maps, core_ids=list(range(N_CORES)))
    outs = [res.results[c]["out_blk"] for c in range(N_CORES)]
    return np.concatenate(outs, axis=0).astype(np.float32)
